# revision 20
# baseline (speedup 1.0000x reference)
"""Trainium2 Bass kernel for nn_MessageAggregationAttention.

Shards B=256 graphs across 8 NeuronCores (32 graphs each). The host does
all data *layout* (gather / pad / transpose / cast); every FLOP of the
model (projections, attention, FFN) runs on device.

Host prep per core:
  - xqT  [128, 3072] : Q token slab, feature-major, f32 (+ out-proj bias
    and folded Wo@bv added in, for the residual spine) and a bf16 copy
    for the Q projection.
  - xkT  [128, 12288]: incoming-message rows gathered on host
    (edge_attr[incoming_edges_list]), zero-padded to LK=384 per graph,
    transposed, bf16. This replaces the baseline's 96 serial INDIRECT1D
    gathers (~105us of GpSimd descriptor processing) with plain DMA.
  - The key bias bk is dropped exactly: softmax is invariant to the
    per-query shift q.bk. Zero-padded K columns then produce logits==0,
    exp==1, so the padded-slot pollution of the softmax denominator is
    exactly (384 - cnt_k); the kernel subtracts it (no mask table).

Device per graph (all matmuls bf16, f32 PSUM):
  - K/V projections from the resident xkT slab.
  - Logits per (key-tile, head) with 32-row PE tiles at partition offset
    32h (no zero-blocked Q weights; Q projection is 6 natural matmuls).
  - Exp on Scalar (no bias operand), denominator via ones[128,32]
    matmuls whose replicated output doubles as the partition broadcast
    for the normalization (no reciprocal broadcast step).
  - Out-proj, residual add, then a batched FFN and direct feature-major
    store; the host transposes/compacts the dense output.
"""

import math

import ml_dtypes
import numpy as np

import concourse.bass as bass
import concourse.mybir as mybir
from concourse import bacc
from concourse.bass_utils import run_bass_kernel_spmd
from concourse.tile import TileContext

B, E, M, H, NH = 256, 16384, 65536, 128, 4
HD = H // NH               # 32
LQ, LK = 96, 384
NCORES = 8
G = B // NCORES            # 32 graphs per core
QS = G * LQ                # 3072 query slots per core
KS = G * LK                # 12288 key slots per core
NQB = QS // 512            # 6 query blocks

f32 = mybir.dt.float32
bf16 = mybir.dt.bfloat16

AFT = mybir.ActivationFunctionType
ALU = mybir.AluOpType

LAST_RESULTS = None
TRACE = False
TRACE_KW = {}


def _build_program():
    nc = bacc.Bacc("TRN2")

    xkT_d = nc.dram_tensor("xkT", [H, KS], bf16, kind="ExternalInput")
    xqbf_d = nc.dram_tensor("xqbf", [H, QS], bf16, kind="ExternalInput")
    xqr_d = nc.dram_tensor("xqr", [H, QS], f32, kind="ExternalInput")
    wqTz_d = nc.dram_tensor("wqTz", [H, 4 * H], bf16, kind="ExternalInput")
    wkT_d = nc.dram_tensor("wkT", [H, H], bf16, kind="ExternalInput")
    wvT_d = nc.dram_tensor("wvT", [H, H], bf16, kind="ExternalInput")
    woT_d = nc.dram_tensor("woT", [H, H], bf16, kind="ExternalInput")
    w1T_d = nc.dram_tensor("w1T", [H, 2 * H], bf16, kind="ExternalInput")
    w2T_d = nc.dram_tensor("w2T", [2 * H, H], bf16, kind="ExternalInput")
    bq_d = nc.dram_tensor("bqz", [H, 4], f32, kind="ExternalInput")
    b1_d = nc.dram_tensor("b1c", [H, 2], f32, kind="ExternalInput")
    b2_d = nc.dram_tensor("b2c", [H, 1], f32, kind="ExternalInput")
    nnp_d = nc.dram_tensor("negnp", [H, G], f32, kind="ExternalInput")

    out_d = nc.dram_tensor("out", [H, QS], f32, kind="ExternalOutput")

    with TileContext(nc) as tc:
        with (
            tc.tile_pool(name="const", bufs=1) as constp,
            tc.tile_pool(name="kv", bufs=5) as kvp,
            tc.tile_pool(name="exp", bufs=6) as expp,
            tc.tile_pool(name="sm", bufs=3) as smp,
            tc.tile_pool(name="ffn", bufs=2) as ffnp,
            tc.tile_pool(name="ps_big", bufs=2, space="PSUM") as ps_bigp,
            tc.tile_pool(name="ps_kv", bufs=1, space="PSUM") as ps_kvp,
            tc.tile_pool(name="ps_lg", bufs=2, space="PSUM") as ps_lgp,
            tc.tile_pool(name="ps_att", bufs=2, space="PSUM") as ps_attp,
        ):
            ones32 = constp.tile([128, 32], bf16)
            nc.vector.memset(ones32[:], 1.0)

            def _load(shape, dram, dt=f32):
                t = constp.tile(shape, dt, tag=dram.name, name=dram.name + "_sb")
                nc.sync.dma_start(out=t[:], in_=dram[:])
                return t

            wqTz = _load([H, 4 * H], wqTz_d, bf16)
            wkT = _load([H, H], wkT_d, bf16)
            wvT = _load([H, H], wvT_d, bf16)
            woT = _load([H, H], woT_d, bf16)
            w1T = _load([H, 2 * H], w1T_d, bf16)
            w2T_a = constp.tile([128, H], bf16, tag="w2Ta")
            w2T_b = constp.tile([128, H], bf16, tag="w2Tb")
            nc.sync.dma_start(out=w2T_a[:], in_=w2T_d[0:128, :])
            nc.sync.dma_start(out=w2T_b[:], in_=w2T_d[128:256, :])
            bqz = _load([H, 4], bq_d)
            b1c = _load([H, 2], b1_d)
            b2c = _load([H, 1], b2_d)
            negnp = _load([H, G], nnp_d)

            # Input slabs: spread dma_start descriptor generation across
            # engine queues (it costs ~0.6us serial per call on one queue)
            # and order chunks so wave-0 consumers land first.
            xkT = constp.tile([128, KS], bf16, tag="xkT", name="xkT")
            xqbf = constp.tile([128, QS], bf16, tag="xqbf", name="xqbf")
            xqr = constp.tile([128, QS], f32, tag="xqr", name="xqr")

            def _chunk(eng, dst, src, c0, c1):
                eng.dma_start(out=dst[:, c0:c1], in_=src[:, c0:c1])

            _chunk(nc.scalar, xqbf, xqbf_d, 0, 512)
            _chunk(nc.scalar, xkT, xkT_d, 0, 768)
            _chunk(nc.gpsimd, xkT, xkT_d, 768, 2304)
            _chunk(nc.gpsimd, xqbf, xqbf_d, 512, 1792)
            for c0 in range(2304, KS, 1728):
                _chunk(nc.sync, xkT, xkT_d, c0, min(c0 + 1728, KS))
            _chunk(nc.sync, xqbf, xqbf_d, 1792, QS)
            _chunk(nc.sync, xqr, xqr_d, 0, 1024)
            _chunk(nc.sync, xqr, xqr_d, 1024, 2048)
            _chunk(nc.gpsimd, xqr, xqr_d, 2048, QS)

            qTz = constp.tile([128, 4, QS], bf16, tag="qTz", name="qTz")
            ar = constp.tile([128, QS], f32, tag="ar", name="ar")

            # ---- stage-pipelined emission ----
            # wave w: qproj(w), kv(w), logits+exp(w-2), ctx+den(w-4),
            # norm+outproj(w-5); FFN blocks interleave once their ar
            # columns are final. Stages hand off through SBUF tiles so
            # the in-order engine queues never wait on work issued in
            # the same wave.
            kT_g, v_g, ex_g, exs_g, att_g = {}, {}, {}, {}, {}

            def emit_qproj(blk):
                sl = slice(blk * 512, (blk + 1) * 512)
                for h in range(4):
                    psq = ps_bigp.tile([128, 512], f32, tag="big", name="psq")
                    nc.tensor.matmul(
                        out=psq[:], lhsT=wqTz[:, h * 128 : (h + 1) * 128],
                        rhs=xqbf[:, sl], start=True, stop=True,
                    )
                    if h < 2:
                        nc.scalar.activation(
                            out=qTz[:, h, sl], in_=psq[:], func=AFT.Identity,
                            bias=bqz[:, h : h + 1],
                        )
                    else:
                        nc.vector.tensor_scalar_add(
                            out=qTz[:, h, sl], in0=psq[:],
                            scalar1=bqz[:, h : h + 1],
                        )

            def emit_kv(g):
                ksl = slice(g * LK, (g + 1) * LK)
                psk = ps_kvp.tile([128, LK], f32, tag="psk", name="psk")
                nc.tensor.matmul(
                    out=psk[:], lhsT=wkT[:], rhs=xkT[:, ksl],
                    start=True, stop=True,
                )
                kT = kvp.tile([128, LK], bf16, tag="kT", name="kT", bufs=5)
                nc.scalar.activation(out=kT[:], in_=psk[:], func=AFT.Identity)
                psv = ps_kvp.tile([128, LK], f32, tag="psv", name="psv")
                for t in range(3):
                    nc.tensor.matmul(
                        out=psv[:, t * 128 : (t + 1) * 128],
                        lhsT=xkT[:, g * LK + t * 128 : g * LK + (t + 1) * 128],
                        rhs=wvT[:],
                        start=True, stop=True, skip_group_check=True,
                    )
                v = kvp.tile([128, LK], bf16, tag="v", name="v", bufs=7)
                nc.vector.tensor_copy(out=v[:], in_=psv[:])
                kT_g[g] = kT
                v_g[g] = v

            def emit_lgx(g):
                """logits + exp + exp-sum for graph g"""
                kT = kT_g.pop(g)
                qsl = slice(g * LQ, (g + 1) * LQ)
                exl = []
                lgl = []
                for t in range(3):
                    lgp = ps_lgp.tile([128, 4 * LQ], f32, tag="lg", name="lgp")
                    nc.tensor.matmul(
                        out=lgp[:],
                        lhsT=kT[:, t * 128 : (t + 1) * 128],
                        rhs=qTz[:, :, qsl],
                        start=True, stop=True,
                    )
                    lgl.append(lgp)
                    ex = expp.tile([128, 4 * LQ], bf16, tag="ex", name="ex",
                                   bufs=10)
                    nc.scalar.activation(out=ex[:], in_=lgp[:], func=AFT.Exp)
                    exl.append(ex)
                exs = expp.tile([128, 4 * LQ], bf16, tag="exs", name="exs",
                                bufs=4)
                nc.gpsimd.tensor_add(out=exs[:], in0=exl[0][:], in1=exl[1][:])
                nc.gpsimd.tensor_add(out=exs[:], in0=exs[:], in1=exl[2][:])
                ex_g[g] = exl
                exs_g[g] = exs

            def emit_cd(g):
                """ctx + denominator matmuls for graph g"""
                v = v_g.pop(g)
                exl = ex_g.pop(g)
                exs = exs_g.pop(g)
                att = ps_attp.tile([128, 192], f32, tag="att", name="att")
                for t in range(3):
                    for h in range(4):
                        nc.tensor.matmul(
                            out=att[32 * h : 32 * (h + 1), 0:LQ],
                            lhsT=v[:, t * 128 + 32 * h : t * 128 + 32 * (h + 1)],
                            rhs=exl[t][:, h * LQ : (h + 1) * LQ],
                            start=(t == 0), stop=(t == 2),
                            skip_group_check=True, tile_position=(0, 32 * h),
                        )
                # denominator, replicated to each head's 32 partitions
                for h in range(4):
                    nc.tensor.matmul(
                        out=att[32 * h : 32 * (h + 1), LQ : 2 * LQ],
                        lhsT=ones32[:],
                        rhs=exs[:, h * LQ : (h + 1) * LQ],
                        start=True, stop=True, skip_group_check=True,
                        tile_position=(0, 32 * h),
                    )
                att_g[g] = att

            def emit_nrm(g):
                """normalize + out-proj + residual for graph g"""
                att = att_g.pop(g)
                qsl = slice(g * LQ, (g + 1) * LQ)
                dsb = smp.tile([128, LQ], f32, tag="dsb", name="dsb")
                nc.vector.tensor_scalar_add(
                    out=dsb[:], in0=att[:, LQ : 2 * LQ],
                    scalar1=negnp[:, g : g + 1],
                )
                rden = smp.tile([128, LQ], f32, tag="rden", name="rden")
                nc.vector.reciprocal_approx_fast(out=rden[:], in_=dsb[:])
                ctxn = smp.tile([128, LQ], bf16, tag="ctxn", name="ctxn")
                nc.vector.tensor_mul(out=ctxn[:], in0=att[:, 0:LQ], in1=rden[:])
                po = ps_lgp.tile([128, 4 * LQ], f32, tag="lg", name="po")
                nc.tensor.matmul(
                    out=po[:, 0:LQ], lhsT=woT[:], rhs=ctxn[:],
                    start=True, stop=True, skip_group_check=True,
                )
                nc.vector.tensor_add(
                    out=ar[:, qsl], in0=po[:, 0:LQ], in1=xqr[:, qsl],
                )

            ffn_state = {}

            def emit_ffn_a(key):
                c0, c1 = key
                n = c1 - c0
                sl = slice(c0, c1)
                arb = ffnp.tile([128, 512], bf16, tag="arb", name="arb")
                nc.vector.tensor_copy(out=arb[:, 0:n], in_=ar[:, sl])
                pa = ps_bigp.tile([128, 512], f32, tag="big", name="pa")
                nc.tensor.matmul(
                    out=pa[:, 0:n], lhsT=w1T[:, 0:128], rhs=arb[:, 0:n],
                    start=True, stop=True, skip_group_check=True,
                )
                ra = ffnp.tile([128, 512], bf16, tag="ra", name="ra")
                nc.scalar.activation(
                    out=ra[:, 0:n], in_=pa[:, 0:n], func=AFT.Relu,
                    bias=b1c[:, 0:1],
                )
                ffn_state[key] = (arb, ra)

            def emit_ffn_b(key, tail=False):
                c0, c1 = key
                n = c1 - c0
                sl = slice(c0, c1)
                arb, ra = ffn_state.pop(key)
                pb = ps_bigp.tile([128, 512], f32, tag="big", name="pb")
                nc.tensor.matmul(
                    out=pb[:, 0:n], lhsT=w1T[:, 128:256], rhs=arb[:, 0:n],
                    start=True, stop=True, skip_group_check=True,
                )
                rb = ffnp.tile([128, 512], bf16, tag="rb", name="rb")
                nc.vector.tensor_scalar(
                    out=rb[:, 0:n], in0=pb[:, 0:n], scalar1=b1c[:, 1:2],
                    scalar2=0.0, op0=ALU.add, op1=ALU.max,
                )
                p2 = ps_bigp.tile([128, 512], f32, tag="big", name="p2")
                nc.tensor.matmul(
                    out=p2[:, 0:n], lhsT=w2T_a[:], rhs=ra[:, 0:n],
                    start=True, stop=False, skip_group_check=True,
                )
                nc.tensor.matmul(
                    out=p2[:, 0:n], lhsT=w2T_b[:], rhs=rb[:, 0:n],
                    start=False, stop=True, skip_group_check=True,
                )
                f2 = ffnp.tile([128, 512], f32, tag="f2", name="f2")
                nc.scalar.activation(
                    out=f2[:, 0:n], in_=p2[:, 0:n], func=AFT.Identity,
                    bias=b2c[:, 0:1],
                )
                if tail:
                    nc.vector.tensor_add(
                        out=ar[:, sl], in0=f2[:, 0:n], in1=ar[:, sl])
                    nc.scalar.dma_start(out=out_d[:, sl], in_=ar[:, sl])
                else:
                    nc.gpsimd.tensor_add(
                        out=ar[:, sl], in0=f2[:, 0:n], in1=ar[:, sl])
                    nc.sync.dma_start(out=out_d[:, sl], in_=ar[:, sl])

            FFN_A = {12: (0, 512), 17: (512, 1024), 22: (1024, 1536),
                     28: (1536, 2048), 34: (2048, 2560),
                     36: (2560, 2816), 37: (2816, 3072)}
            FFN_B = {13: (0, 512), 18: (512, 1024), 23: (1024, 1536),
                     29: (1536, 2048), 35: (2048, 2560),
                     37: (2560, 2816), 38: (2816, 3072)}
            for w in range(G + 7):
                if w < NQB:
                    emit_qproj(w)
                if w < G:
                    emit_kv(w)
                if 2 <= w < G + 2:
                    emit_lgx(w - 2)
                if 4 <= w < G + 4:
                    emit_cd(w - 4)
                if 5 <= w < G + 5:
                    emit_nrm(w - 5)
                if w in FFN_A:
                    emit_ffn_a(FFN_A[w])
                if w in FFN_B:
                    emit_ffn_b(FFN_B[w], tail=(w >= 37))
    nc.finalize()
    return nc


_NC_CACHE = None


def kernel(edge_index, edge_attr, incoming_edges_list, incoming_edges_batch,
           edge_batch, in_proj_w, in_proj_b, out_proj_w, out_proj_b,
           w1, b1, w2, b2):
    global _NC_CACHE, LAST_RESULTS

    edge_attr = np.asarray(edge_attr, np.float32)
    edge_batch = np.asarray(edge_batch, np.int64)
    incoming_edges_list = np.asarray(incoming_edges_list, np.int64)
    incoming_edges_batch = np.asarray(incoming_edges_batch, np.int64)

    cnt_q = np.bincount(edge_batch, minlength=B)
    st_q = np.zeros(B + 1, np.int64)
    np.cumsum(cnt_q, out=st_q[1:])
    cnt_k = np.bincount(incoming_edges_batch, minlength=B)
    st_k = np.zeros(B + 1, np.int64)
    np.cumsum(cnt_k, out=st_k[1:])
    assert cnt_q.max() <= LQ and cnt_k.max() <= LK

    xpad = np.zeros((E + LQ, H), np.float32)
    xpad[:E] = edge_attr

    # Q slabs: contiguous rows from each graph's first edge (may run into
    # the next graph's rows — those query slots are never read back)
    pos_q = np.arange(LQ)[None, :]
    slab_rows = st_q[:B, None] + pos_q                     # [B, LQ]

    # K gather rows, zero-row (index E) for padded slots
    pos_k = np.arange(LK)[None, :]
    valid = pos_k < cnt_k[:, None]
    flat = st_k[:B, None] + np.minimum(pos_k, cnt_k[:, None] - 1)
    gath = np.where(valid, incoming_edges_list[flat], E)   # [B, LK]

    s = 1.0 / math.sqrt(HD)
    wq, wk, wv = in_proj_w[:H], in_proj_w[H : 2 * H], in_proj_w[2 * H :]
    bq, bv = in_proj_b[:H], in_proj_b[2 * H :]
    # bk is dropped exactly: softmax is invariant to the per-query shift
    # q.bk added uniformly across a query's keys.
    boc = out_proj_b + out_proj_w @ bv

    wqT = np.ascontiguousarray((wq * s).T, np.float32)
    wqTz = np.zeros((H, 4 * H), np.float32)
    bqz = np.zeros((H, 4), np.float32)
    for h in range(4):
        wqTz[:, h * H + 32 * h : h * H + 32 * (h + 1)] = \
            wqT[:, 32 * h : 32 * (h + 1)]
        bqz[32 * h : 32 * (h + 1), h] = (bq * s)[32 * h : 32 * (h + 1)]

    bft = ml_dtypes.bfloat16
    shared = dict(
        wqTz=np.ascontiguousarray(wqTz.astype(bft)),
        bqz=np.ascontiguousarray(bqz),
        wkT=np.ascontiguousarray(wk.T.astype(bft)),
        wvT=np.ascontiguousarray(wv.T.astype(bft)),
        woT=np.ascontiguousarray(out_proj_w.T.astype(bft)),
        w1T=np.ascontiguousarray(w1.T.astype(bft)),
        w2T=np.ascontiguousarray(w2.T.astype(bft)),
        b1c=np.ascontiguousarray(b1.reshape(2, H).T, np.float32),
        b2c=np.ascontiguousarray(b2[:, None], np.float32),
    )

    in_maps = []
    for c in range(NCORES):
        gs = slice(c * G, (c + 1) * G)
        xq = xpad[slab_rows[gs].reshape(-1)]               # [QS, H] f32
        xk = xpad[gath[gs].reshape(-1)]                    # [KS, H] f32
        negnp_c = np.broadcast_to(
            -(LK - cnt_k[gs]).astype(np.float32), (H, G))
        in_maps.append(dict(
            shared,
            xqr=np.ascontiguousarray(xq.T) + boc[:, None].astype(np.float32),
            xqbf=np.ascontiguousarray(xq.T.astype(bft)),
            xkT=np.ascontiguousarray(xk.T.astype(bft)),
            negnp=np.ascontiguousarray(negnp_c),
        ))

    if _NC_CACHE is None:
        _NC_CACHE = _build_program()
    res = run_bass_kernel_spmd(
        _NC_CACHE, in_maps, core_ids=list(range(NCORES)),
        trace=TRACE, **TRACE_KW,
    )
    LAST_RESULTS = res

    # compact: edge e lives at dense col (g_local*LQ + pos) of its core
    eb = edge_batch
    g_local = (eb % G).astype(np.int64)
    pos = np.arange(E) - st_q[eb]
    slot = g_local * LQ + pos
    out_full = np.empty((E, H), np.float32)
    for c in range(NCORES):
        sel = (eb // G) == c
        out_full[sel] = res.results[c]["out"].T[slot[sel]]
    return out_full


# revision 21
# speedup vs baseline: 1.3350x; 1.3350x over previous
"""Trainium2 Bass kernel for nn_MessageAggregationAttention.

Shards B=256 graphs across 8 NeuronCores (32 graphs each). The host does
all data *layout* (gather / pad / transpose / cast); every FLOP of the
model (projections, attention, FFN) runs on device.

Host prep per core:
  - xqT  [128, 3072] : Q token slab, feature-major, f32 (+ out-proj bias
    and folded Wo@bv added in, for the residual spine) and a bf16 copy
    for the Q projection.
  - xkT  [128, 12288]: incoming-message rows gathered on host
    (edge_attr[incoming_edges_list]), zero-padded to LK=384 per graph,
    transposed, bf16. This replaces the baseline's 96 serial INDIRECT1D
    gathers (~105us of GpSimd descriptor processing) with plain DMA.
  - The key bias bk is dropped exactly: softmax is invariant to the
    per-query shift q.bk. Zero-padded K columns then produce logits==0,
    exp==1, so the padded-slot pollution of the softmax denominator is
    exactly (384 - cnt_k); the kernel subtracts it (no mask table).

Device per graph (all matmuls bf16, f32 PSUM):
  - K/V projections from the resident xkT slab.
  - Logits per (key-tile, head) with 32-row PE tiles at partition offset
    32h (no zero-blocked Q weights; Q projection is 6 natural matmuls).
  - Exp on Scalar (no bias operand), denominator via ones[128,32]
    matmuls whose replicated output doubles as the partition broadcast
    for the normalization (no reciprocal broadcast step).
  - Out-proj, residual add, then a batched FFN and direct feature-major
    store; the host transposes/compacts the dense output.
"""

import math

import ml_dtypes
import numpy as np

import concourse.bass as bass
import concourse.mybir as mybir
from concourse import bacc
from concourse.bass_utils import run_bass_kernel_spmd
from concourse.tile import TileContext

B, E, M, H, NH = 256, 16384, 65536, 128, 4
HD = H // NH               # 32
LQ, LK = 96, 384
NCORES = 8
G = B // NCORES            # 32 graphs per core
QS = G * LQ                # 3072 query slots per core
KS = G * LK                # 12288 key slots per core
NQB = QS // 512            # 6 query blocks

f32 = mybir.dt.float32
bf16 = mybir.dt.bfloat16

AFT = mybir.ActivationFunctionType
ALU = mybir.AluOpType

LAST_RESULTS = None
TRACE = False
TRACE_KW = {}


def _build_program():
    nc = bacc.Bacc("TRN2")

    xkT_d = nc.dram_tensor("xkT", [H, KS], bf16, kind="ExternalInput")
    xqbf_d = nc.dram_tensor("xqbf", [H, QS], bf16, kind="ExternalInput")
    xqr_d = nc.dram_tensor("xqr", [H, QS], f32, kind="ExternalInput")
    wqTz_d = nc.dram_tensor("wqTz", [H, 4 * H], bf16, kind="ExternalInput")
    wkT_d = nc.dram_tensor("wkT", [H, H], bf16, kind="ExternalInput")
    wvT_d = nc.dram_tensor("wvT", [H, H], bf16, kind="ExternalInput")
    woT_d = nc.dram_tensor("woT", [H, H], bf16, kind="ExternalInput")
    w1T_d = nc.dram_tensor("w1T", [H, 2 * H], bf16, kind="ExternalInput")
    w2T_d = nc.dram_tensor("w2T", [2 * H, H], bf16, kind="ExternalInput")
    bq_d = nc.dram_tensor("bqz", [H, 4], f32, kind="ExternalInput")
    b1_d = nc.dram_tensor("b1c", [H, 2], f32, kind="ExternalInput")
    b2_d = nc.dram_tensor("b2c", [H, 1], f32, kind="ExternalInput")
    nnp_d = nc.dram_tensor("negnp", [H, G], f32, kind="ExternalInput")

    out_d = nc.dram_tensor("out", [H, QS], f32, kind="ExternalOutput")

    with TileContext(nc) as tc:
        with (
            tc.tile_pool(name="const", bufs=1) as constp,
            tc.tile_pool(name="kv", bufs=5) as kvp,
            tc.tile_pool(name="exp", bufs=6) as expp,
            tc.tile_pool(name="sm", bufs=3) as smp,
            tc.tile_pool(name="ffn", bufs=2) as ffnp,
            tc.tile_pool(name="ps_big", bufs=2, space="PSUM") as ps_bigp,
            tc.tile_pool(name="ps_kv", bufs=1, space="PSUM") as ps_kvp,
            tc.tile_pool(name="ps_lg", bufs=2, space="PSUM") as ps_lgp,
            tc.tile_pool(name="ps_att", bufs=2, space="PSUM") as ps_attp,
        ):
            ones32 = constp.tile([128, 32], bf16)
            nc.vector.memset(ones32[:], 1.0)

            def _load(shape, dram, dt=f32):
                t = constp.tile(shape, dt, tag=dram.name, name=dram.name + "_sb")
                nc.sync.dma_start(out=t[:], in_=dram[:])
                return t

            wqTz = _load([H, 4 * H], wqTz_d, bf16)
            wkT = _load([H, H], wkT_d, bf16)
            wvT = _load([H, H], wvT_d, bf16)
            woT = _load([H, H], woT_d, bf16)
            w1T = _load([H, 2 * H], w1T_d, bf16)
            w2T_a = constp.tile([128, H], bf16, tag="w2Ta")
            w2T_b = constp.tile([128, H], bf16, tag="w2Tb")
            nc.sync.dma_start(out=w2T_a[:], in_=w2T_d[0:128, :])
            nc.sync.dma_start(out=w2T_b[:], in_=w2T_d[128:256, :])
            bqz = _load([H, 4], bq_d)
            b1c = _load([H, 2], b1_d)
            b2c = _load([H, 1], b2_d)
            negnp = _load([H, G], nnp_d)

            # Input slabs: spread dma_start descriptor generation across
            # engine queues (it costs ~0.6us serial per call on one queue)
            # and order chunks so wave-0 consumers land first.
            xkT = constp.tile([128, KS], bf16, tag="xkT", name="xkT")
            xqbf = constp.tile([128, QS], bf16, tag="xqbf", name="xqbf")
            xqr = constp.tile([128, QS], f32, tag="xqr", name="xqr")

            def _chunk(eng, dst, src, c0, c1):
                eng.dma_start(out=dst[:, c0:c1], in_=src[:, c0:c1])

            _chunk(nc.scalar, xqbf, xqbf_d, 0, 512)
            _chunk(nc.scalar, xkT, xkT_d, 0, 768)
            _chunk(nc.gpsimd, xkT, xkT_d, 768, 2304)
            _chunk(nc.gpsimd, xqbf, xqbf_d, 512, 1792)
            for c0 in range(2304, KS, 1728):
                _chunk(nc.sync, xkT, xkT_d, c0, min(c0 + 1728, KS))
            _chunk(nc.sync, xqbf, xqbf_d, 1792, QS)
            _chunk(nc.sync, xqr, xqr_d, 0, 1024)
            _chunk(nc.sync, xqr, xqr_d, 1024, 2048)
            _chunk(nc.gpsimd, xqr, xqr_d, 2048, QS)

            qTz = constp.tile([128, 4, QS], bf16, tag="qTz", name="qTz")
            ar = constp.tile([128, QS], f32, tag="ar", name="ar")

            # ---- stage-pipelined emission ----
            # wave w: qproj(w), kv(w), logits+exp(w-2), ctx+den(w-4),
            # norm+outproj(w-5); FFN blocks interleave once their ar
            # columns are final. Stages hand off through SBUF tiles so
            # the in-order engine queues never wait on work issued in
            # the same wave.
            kT_g, v_g, ex_g, exs_g, att_g = {}, {}, {}, {}, {}

            def emit_qproj(blk):
                sl = slice(blk * 512, (blk + 1) * 512)
                for h in range(4):
                    psq = ps_bigp.tile([128, 512], f32, tag="big", name="psq")
                    nc.tensor.matmul(
                        out=psq[:], lhsT=wqTz[:, h * 128 : (h + 1) * 128],
                        rhs=xqbf[:, sl], start=True, stop=True,
                    )
                    if h < 2:
                        nc.scalar.activation(
                            out=qTz[:, h, sl], in_=psq[:], func=AFT.Identity,
                            bias=bqz[:, h : h + 1],
                        )
                    else:
                        nc.vector.tensor_scalar_add(
                            out=qTz[:, h, sl], in0=psq[:],
                            scalar1=bqz[:, h : h + 1],
                        )

            def emit_kv(g):
                ksl = slice(g * LK, (g + 1) * LK)
                psk = ps_kvp.tile([128, LK], f32, tag="psk", name="psk")
                nc.tensor.matmul(
                    out=psk[:], lhsT=wkT[:], rhs=xkT[:, ksl],
                    start=True, stop=True,
                )
                kT = kvp.tile([128, LK], bf16, tag="kT", name="kT", bufs=5)
                nc.scalar.activation(out=kT[:], in_=psk[:], func=AFT.Identity)
                psv = ps_kvp.tile([128, LK], f32, tag="psv", name="psv")
                for t in range(3):
                    nc.tensor.matmul(
                        out=psv[:, t * 128 : (t + 1) * 128],
                        lhsT=xkT[:, g * LK + t * 128 : g * LK + (t + 1) * 128],
                        rhs=wvT[:],
                        start=True, stop=True, skip_group_check=True,
                    )
                v = kvp.tile([128, LK], bf16, tag="v", name="v", bufs=7)
                nc.vector.tensor_copy(out=v[:], in_=psv[:])
                kT_g[g] = kT
                v_g[g] = v

            def emit_lgx(g):
                """logits + exp + exp-sum for graph g"""
                kT = kT_g.pop(g)
                qsl = slice(g * LQ, (g + 1) * LQ)
                exl = []
                lgl = []
                for t in range(3):
                    lgp = ps_lgp.tile([128, 4 * LQ], f32, tag="lg", name="lgp")
                    nc.tensor.matmul(
                        out=lgp[:],
                        lhsT=kT[:, t * 128 : (t + 1) * 128],
                        rhs=qTz[:, :, qsl],
                        start=True, stop=True,
                    )
                    lgl.append(lgp)
                    ex = expp.tile([128, 4 * LQ], bf16, tag="ex", name="ex",
                                   bufs=10)
                    nc.scalar.activation(out=ex[:], in_=lgp[:], func=AFT.Exp)
                    exl.append(ex)
                exs = expp.tile([128, 4 * LQ], bf16, tag="exs", name="exs",
                                bufs=4)
                nc.gpsimd.tensor_add(out=exs[:], in0=exl[0][:], in1=exl[1][:])
                nc.vector.tensor_add(out=exs[:], in0=exs[:], in1=exl[2][:])
                ex_g[g] = exl
                exs_g[g] = exs

            def emit_cd(g):
                """ctx + denominator matmuls for graph g"""
                v = v_g.pop(g)
                exl = ex_g.pop(g)
                exs = exs_g.pop(g)
                att = ps_attp.tile([128, 192], f32, tag="att", name="att")
                for t in range(3):
                    for h in range(4):
                        nc.tensor.matmul(
                            out=att[32 * h : 32 * (h + 1), 0:LQ],
                            lhsT=v[:, t * 128 + 32 * h : t * 128 + 32 * (h + 1)],
                            rhs=exl[t][:, h * LQ : (h + 1) * LQ],
                            start=(t == 0), stop=(t == 2),
                            skip_group_check=True, tile_position=(0, 32 * h),
                        )
                # denominator, replicated to each head's 32 partitions
                for h in range(4):
                    nc.tensor.matmul(
                        out=att[32 * h : 32 * (h + 1), LQ : 2 * LQ],
                        lhsT=ones32[:],
                        rhs=exs[:, h * LQ : (h + 1) * LQ],
                        start=True, stop=True, skip_group_check=True,
                        tile_position=(0, 32 * h),
                    )
                att_g[g] = att

            def emit_nrm(g):
                """normalize + out-proj + residual for graph g"""
                att = att_g.pop(g)
                qsl = slice(g * LQ, (g + 1) * LQ)
                dsb = smp.tile([128, LQ], f32, tag="dsb", name="dsb")
                nc.vector.tensor_scalar_add(
                    out=dsb[:], in0=att[:, LQ : 2 * LQ],
                    scalar1=negnp[:, g : g + 1],
                )
                rden = smp.tile([128, LQ], f32, tag="rden", name="rden")
                nc.vector.reciprocal_approx_fast(out=rden[:], in_=dsb[:])
                ctxn = smp.tile([128, LQ], bf16, tag="ctxn", name="ctxn")
                nc.vector.tensor_mul(out=ctxn[:], in0=att[:, 0:LQ], in1=rden[:])
                po = ps_lgp.tile([128, 4 * LQ], f32, tag="lg", name="po")
                nc.tensor.matmul(
                    out=po[:, 0:LQ], lhsT=woT[:], rhs=ctxn[:],
                    start=True, stop=True, skip_group_check=True,
                )
                nc.vector.tensor_add(
                    out=ar[:, qsl], in0=po[:, 0:LQ], in1=xqr[:, qsl],
                )

            ffn_state = {}

            def emit_ffn_a(key):
                c0, c1 = key
                n = c1 - c0
                sl = slice(c0, c1)
                arb = ffnp.tile([128, 512], bf16, tag="arb", name="arb")
                nc.vector.tensor_copy(out=arb[:, 0:n], in_=ar[:, sl])
                pa = ps_bigp.tile([128, 512], f32, tag="big", name="pa")
                nc.tensor.matmul(
                    out=pa[:, 0:n], lhsT=w1T[:, 0:128], rhs=arb[:, 0:n],
                    start=True, stop=True, skip_group_check=True,
                )
                ra = ffnp.tile([128, 512], bf16, tag="ra", name="ra")
                nc.scalar.activation(
                    out=ra[:, 0:n], in_=pa[:, 0:n], func=AFT.Relu,
                    bias=b1c[:, 0:1],
                )
                ffn_state[key] = (arb, ra)

            def emit_ffn_b(key, tail=False):
                c0, c1 = key
                n = c1 - c0
                sl = slice(c0, c1)
                arb, ra = ffn_state.pop(key)
                pb = ps_bigp.tile([128, 512], f32, tag="big", name="pb")
                nc.tensor.matmul(
                    out=pb[:, 0:n], lhsT=w1T[:, 128:256], rhs=arb[:, 0:n],
                    start=True, stop=True, skip_group_check=True,
                )
                rb = ffnp.tile([128, 512], bf16, tag="rb", name="rb")
                nc.vector.tensor_scalar(
                    out=rb[:, 0:n], in0=pb[:, 0:n], scalar1=b1c[:, 1:2],
                    scalar2=0.0, op0=ALU.add, op1=ALU.max,
                )
                p2 = ps_bigp.tile([128, 512], f32, tag="big", name="p2")
                nc.tensor.matmul(
                    out=p2[:, 0:n], lhsT=w2T_a[:], rhs=ra[:, 0:n],
                    start=True, stop=False, skip_group_check=True,
                )
                nc.tensor.matmul(
                    out=p2[:, 0:n], lhsT=w2T_b[:], rhs=rb[:, 0:n],
                    start=False, stop=True, skip_group_check=True,
                )
                f2 = ffnp.tile([128, 512], f32, tag="f2", name="f2")
                nc.scalar.activation(
                    out=f2[:, 0:n], in_=p2[:, 0:n], func=AFT.Identity,
                    bias=b2c[:, 0:1],
                )
                if tail:
                    nc.vector.tensor_add(
                        out=ar[:, sl], in0=f2[:, 0:n], in1=ar[:, sl])
                    nc.scalar.dma_start(out=out_d[:, sl], in_=ar[:, sl])
                else:
                    nc.gpsimd.tensor_add(
                        out=ar[:, sl], in0=f2[:, 0:n], in1=ar[:, sl])
                    nc.sync.dma_start(out=out_d[:, sl], in_=ar[:, sl])

            FFN_A = {12: (0, 512), 17: (512, 1024), 22: (1024, 1536),
                     28: (1536, 2048), 34: (2048, 2560),
                     36: (2560, 2816), 37: (2816, 3072)}
            FFN_B = {13: (0, 512), 18: (512, 1024), 23: (1024, 1536),
                     29: (1536, 2048), 35: (2048, 2560),
                     37: (2560, 2816), 38: (2816, 3072)}
            for w in range(G + 7):
                if w < NQB:
                    emit_qproj(w)
                if w < G:
                    emit_kv(w)
                if 2 <= w < G + 2:
                    emit_lgx(w - 2)
                if 4 <= w < G + 4:
                    emit_cd(w - 4)
                if 5 <= w < G + 5:
                    emit_nrm(w - 5)
                if w in FFN_A:
                    emit_ffn_a(FFN_A[w])
                if w in FFN_B:
                    emit_ffn_b(FFN_B[w], tail=(w >= 37))
    nc.finalize()
    return nc


_NC_CACHE = None


def kernel(edge_index, edge_attr, incoming_edges_list, incoming_edges_batch,
           edge_batch, in_proj_w, in_proj_b, out_proj_w, out_proj_b,
           w1, b1, w2, b2):
    global _NC_CACHE, LAST_RESULTS

    edge_attr = np.asarray(edge_attr, np.float32)
    edge_batch = np.asarray(edge_batch, np.int64)
    incoming_edges_list = np.asarray(incoming_edges_list, np.int64)
    incoming_edges_batch = np.asarray(incoming_edges_batch, np.int64)

    cnt_q = np.bincount(edge_batch, minlength=B)
    st_q = np.zeros(B + 1, np.int64)
    np.cumsum(cnt_q, out=st_q[1:])
    cnt_k = np.bincount(incoming_edges_batch, minlength=B)
    st_k = np.zeros(B + 1, np.int64)
    np.cumsum(cnt_k, out=st_k[1:])
    assert cnt_q.max() <= LQ and cnt_k.max() <= LK

    xpad = np.zeros((E + LQ, H), np.float32)
    xpad[:E] = edge_attr

    # Q slabs: contiguous rows from each graph's first edge (may run into
    # the next graph's rows — those query slots are never read back)
    pos_q = np.arange(LQ)[None, :]
    slab_rows = st_q[:B, None] + pos_q                     # [B, LQ]

    # K gather rows, zero-row (index E) for padded slots
    pos_k = np.arange(LK)[None, :]
    valid = pos_k < cnt_k[:, None]
    flat = st_k[:B, None] + np.minimum(pos_k, cnt_k[:, None] - 1)
    gath = np.where(valid, incoming_edges_list[flat], E)   # [B, LK]

    s = 1.0 / math.sqrt(HD)
    wq, wk, wv = in_proj_w[:H], in_proj_w[H : 2 * H], in_proj_w[2 * H :]
    bq, bv = in_proj_b[:H], in_proj_b[2 * H :]
    # bk is dropped exactly: softmax is invariant to the per-query shift
    # q.bk added uniformly across a query's keys.
    boc = out_proj_b + out_proj_w @ bv

    wqT = np.ascontiguousarray((wq * s).T, np.float32)
    wqTz = np.zeros((H, 4 * H), np.float32)
    bqz = np.zeros((H, 4), np.float32)
    for h in range(4):
        wqTz[:, h * H + 32 * h : h * H + 32 * (h + 1)] = \
            wqT[:, 32 * h : 32 * (h + 1)]
        bqz[32 * h : 32 * (h + 1), h] = (bq * s)[32 * h : 32 * (h + 1)]

    bft = ml_dtypes.bfloat16
    shared = dict(
        wqTz=np.ascontiguousarray(wqTz.astype(bft)),
        bqz=np.ascontiguousarray(bqz),
        wkT=np.ascontiguousarray(wk.T.astype(bft)),
        wvT=np.ascontiguousarray(wv.T.astype(bft)),
        woT=np.ascontiguousarray(out_proj_w.T.astype(bft)),
        w1T=np.ascontiguousarray(w1.T.astype(bft)),
        w2T=np.ascontiguousarray(w2.T.astype(bft)),
        b1c=np.ascontiguousarray(b1.reshape(2, H).T, np.float32),
        b2c=np.ascontiguousarray(b2[:, None], np.float32),
    )

    in_maps = []
    for c in range(NCORES):
        gs = slice(c * G, (c + 1) * G)
        xq = xpad[slab_rows[gs].reshape(-1)]               # [QS, H] f32
        xk = xpad[gath[gs].reshape(-1)]                    # [KS, H] f32
        negnp_c = np.broadcast_to(
            -(LK - cnt_k[gs]).astype(np.float32), (H, G))
        in_maps.append(dict(
            shared,
            xqr=np.ascontiguousarray(xq.T) + boc[:, None].astype(np.float32),
            xqbf=np.ascontiguousarray(xq.T.astype(bft)),
            xkT=np.ascontiguousarray(xk.T.astype(bft)),
            negnp=np.ascontiguousarray(negnp_c),
        ))

    if _NC_CACHE is None:
        _NC_CACHE = _build_program()
    res = run_bass_kernel_spmd(
        _NC_CACHE, in_maps, core_ids=list(range(NCORES)),
        trace=TRACE, **TRACE_KW,
    )
    LAST_RESULTS = res

    # compact: edge e lives at dense col (g_local*LQ + pos) of its core
    eb = edge_batch
    g_local = (eb % G).astype(np.int64)
    pos = np.arange(E) - st_q[eb]
    slot = g_local * LQ + pos
    out_full = np.empty((E, H), np.float32)
    for c in range(NCORES):
        sel = (eb // G) == c
        out_full[sel] = res.results[c]["out"].T[slot[sel]]
    return out_full


# revision 22
# speedup vs baseline: 1.5793x; 1.1829x over previous
"""Trainium2 Bass kernel for nn_MessageAggregationAttention.

Shards B=256 graphs across 8 NeuronCores (32 graphs each). The host does
all data *layout* (gather / pad / transpose / cast); every FLOP of the
model (projections, attention, FFN) runs on device.

Shape specialization: graph sizes are known at kernel() time, so each
core sorts its graphs by key-count and rank-i graphs across cores share
slot i, whose capacities are the across-core maxima: QC[i] query slots
(multiple of 4, <= 96 instead of a flat 96 pad) and KT[i] 128-key tiles
(2 or 3 instead of a flat 3). This removes ~25% of the padded attention
work. The program is built once per shape signature and reused.

Host prep per core:
  - xqr/xqbf [128, QS2]: Q token slab, feature-major (f32 with the
    out-proj bias + folded Wo@bv added for the residual spine; bf16 copy
    for the Q projection).
  - xkT [128, KS2]: incoming-message rows gathered on host
    (edge_attr[incoming_edges_list]), zero-padded per slot, transposed,
    bf16 — replaces 96 serial INDIRECT1D gathers (~105us of GpSimd
    descriptor time) with plain DMA.
  - The key bias bk is dropped exactly (softmax is invariant to the
    per-query shift q.bk); zero-padded K columns then give logits==0,
    exp==1, so the denominator over-counts by exactly npad, which the
    kernel subtracts (no mask table at all).

Device per slot (all matmuls bf16, f32 PSUM), software-pipelined in
waves (kv | logits+exp | ctx+den | norm+outproj, 5 waves deep) so the
in-order engine queues never wait on same-wave work:
  - K/V projections from the resident xkT slab.
  - Logits: zero-blocked qTz (full-128 contraction) per key tile; Exp on
    Scalar with no bias operand.
  - Denominator via ones[128,32] matmuls whose replicated output doubles
    as the partition-broadcast for normalization.
  - Out-proj, residual add; FFN blocks interleave into the wave loop as
    their columns finalize, and the output streams out per block.
"""

import math

import ml_dtypes
import numpy as np

import concourse.bass as bass
import concourse.mybir as mybir
from concourse import bacc
from concourse.bass_utils import run_bass_kernel_spmd
from concourse.tile import TileContext

B, E, M, H, NH = 256, 16384, 65536, 128, 4
HD = H // NH               # 32
LQ, LK = 96, 384           # hard capacity ceilings per graph
NCORES = 8
G = B // NCORES            # 32 graphs per core

f32 = mybir.dt.float32
bf16 = mybir.dt.bfloat16

AFT = mybir.ActivationFunctionType
ALU = mybir.AluOpType

LAST_RESULTS = None
TRACE = False
TRACE_KW = {}


def _build_program(QC, KT):
    QOFF = [0]
    for q in QC:
        QOFF.append(QOFF[-1] + q)
    KOFF = [0]
    for k in KT:
        KOFF.append(KOFF[-1] + 128 * k)
    QS2, KS2 = QOFF[-1], KOFF[-1]

    # FFN blocks of <=512 cols; the last one split in two to drain faster
    blocks = []
    c = 0
    while c < QS2:
        blocks.append((c, min(c + 512, QS2)))
        c = min(c + 512, QS2)
    b0, b1 = blocks.pop()
    if b1 - b0 > 256:
        mid = b0 + ((b1 - b0) // 2 + 3) // 4 * 4
        blocks.append((b0, mid))
        blocks.append((mid, b1))
    else:
        blocks.append((b0, b1))
    # earliest wave per block: its last slot s finishes norm at wave s+5
    ffn_a, ffn_b = {}, {}
    prev_wa = -10
    for (c0, c1) in blocks:
        smax = max(s for s in range(G) if QOFF[s] < c1)
        wa = max(smax + 6, prev_wa + 2)
        prev_wa = wa
        ffn_a.setdefault(wa, []).append((c0, c1))
        ffn_b.setdefault(wa + 1, []).append((c0, c1))
    tail_keys = set(blocks[-2:])
    n_waves = max(G + 7, max(ffn_b) + 1)

    nc = bacc.Bacc("TRN2")

    xkT_d = nc.dram_tensor("xkT", [H, KS2], bf16, kind="ExternalInput")
    xqbf_d = nc.dram_tensor("xqbf", [H, QS2], bf16, kind="ExternalInput")
    xqr_d = nc.dram_tensor("xqr", [H, QS2], f32, kind="ExternalInput")
    wqTz_d = nc.dram_tensor("wqTz", [H, 4 * H], bf16, kind="ExternalInput")
    wkT_d = nc.dram_tensor("wkT", [H, H], bf16, kind="ExternalInput")
    wvT_d = nc.dram_tensor("wvT", [H, H], bf16, kind="ExternalInput")
    woT_d = nc.dram_tensor("woT", [H, H], bf16, kind="ExternalInput")
    w1T_d = nc.dram_tensor("w1T", [H, 2 * H], bf16, kind="ExternalInput")
    w2T_d = nc.dram_tensor("w2T", [2 * H, H], bf16, kind="ExternalInput")
    bq_d = nc.dram_tensor("bqz", [H, 4], f32, kind="ExternalInput")
    b1_d = nc.dram_tensor("b1c", [H, 2], f32, kind="ExternalInput")
    b2_d = nc.dram_tensor("b2c", [H, 1], f32, kind="ExternalInput")
    nnp_d = nc.dram_tensor("negnp", [H, G], f32, kind="ExternalInput")

    out_d = nc.dram_tensor("out", [H, QS2], f32, kind="ExternalOutput")

    with TileContext(nc) as tc:
        with (
            tc.tile_pool(name="const", bufs=1) as constp,
            tc.tile_pool(name="kv", bufs=5) as kvp,
            tc.tile_pool(name="exp", bufs=6) as expp,
            tc.tile_pool(name="sm", bufs=3) as smp,
            tc.tile_pool(name="ffn", bufs=2) as ffnp,
            tc.tile_pool(name="ps_big", bufs=2, space="PSUM") as ps_bigp,
            tc.tile_pool(name="ps_kv", bufs=1, space="PSUM") as ps_kvp,
            tc.tile_pool(name="ps_lg", bufs=2, space="PSUM") as ps_lgp,
            tc.tile_pool(name="ps_att", bufs=2, space="PSUM") as ps_attp,
        ):
            ones32 = constp.tile([128, 32], bf16)
            nc.vector.memset(ones32[:], 1.0)

            def _load(shape, dram, dt=f32):
                t = constp.tile(shape, dt, tag=dram.name, name=dram.name + "_sb")
                nc.sync.dma_start(out=t[:], in_=dram[:])
                return t

            wqTz = _load([H, 4 * H], wqTz_d, bf16)
            wkT = _load([H, H], wkT_d, bf16)
            wvT = _load([H, H], wvT_d, bf16)
            woT = _load([H, H], woT_d, bf16)
            w1T = _load([H, 2 * H], w1T_d, bf16)
            w2T_a = constp.tile([128, H], bf16, tag="w2Ta")
            w2T_b = constp.tile([128, H], bf16, tag="w2Tb")
            nc.sync.dma_start(out=w2T_a[:], in_=w2T_d[0:128, :])
            nc.sync.dma_start(out=w2T_b[:], in_=w2T_d[128:256, :])
            bqz = _load([H, 4], bq_d)
            b1c = _load([H, 2], b1_d)
            b2c = _load([H, 1], b2_d)
            negnp = _load([H, G], nnp_d)

            # Input slabs: spread dma_start descriptor generation across
            # engine queues (~0.6us serial per call) and order chunks so
            # wave-0 consumers land first.
            xkT = constp.tile([128, KS2], bf16, tag="xkT", name="xkT")
            xqbf = constp.tile([128, QS2], bf16, tag="xqbf", name="xqbf")
            xqr = constp.tile([128, QS2], f32, tag="xqr", name="xqr")

            def _chunk(eng, dst, src, c0, c1):
                if c1 > c0:
                    eng.dma_start(out=dst[:, c0:c1], in_=src[:, c0:c1])

            q1 = min(512, QS2)
            q2 = min(1792, QS2)
            _chunk(nc.scalar, xqbf, xqbf_d, 0, q1)
            _chunk(nc.scalar, xkT, xkT_d, 0, KOFF[2])
            _chunk(nc.gpsimd, xkT, xkT_d, KOFF[2], KOFF[6])
            _chunk(nc.gpsimd, xqbf, xqbf_d, q1, q2)
            for s0 in range(6, G, 5):
                _chunk(nc.sync, xkT, xkT_d, KOFF[s0], KOFF[min(s0 + 5, G)])
            _chunk(nc.sync, xqbf, xqbf_d, q2, QS2)
            _chunk(nc.sync, xqr, xqr_d, 0, QS2 // 2 // 4 * 4)
            _chunk(nc.gpsimd, xqr, xqr_d, QS2 // 2 // 4 * 4, QS2)

            qTz = constp.tile([128, 4, QS2], bf16, tag="qTz", name="qTz")
            ar = constp.tile([128, QS2], f32, tag="ar", name="ar")

            def emit_qproj(blk):
                c0 = blk * 512
                c1 = min(c0 + 512, QS2)
                if c0 >= QS2:
                    return
                sl = slice(c0, c1)
                n = c1 - c0
                for h in range(4):
                    psq = ps_bigp.tile([128, 512], f32, tag="big", name="psq")
                    nc.tensor.matmul(
                        out=psq[:, 0:n], lhsT=wqTz[:, h * 128 : (h + 1) * 128],
                        rhs=xqbf[:, sl], start=True, stop=True,
                        skip_group_check=True,
                    )
                    if h < 2:
                        nc.scalar.activation(
                            out=qTz[:, h, sl], in_=psq[:, 0:n],
                            func=AFT.Identity, bias=bqz[:, h : h + 1],
                        )
                    else:
                        nc.vector.tensor_scalar_add(
                            out=qTz[:, h, sl], in0=psq[:, 0:n],
                            scalar1=bqz[:, h : h + 1],
                        )

            kT_g, v_g, ex_g, exs_g, att_g = {}, {}, {}, {}, {}

            def emit_kv(g):
                kw = 128 * KT[g]
                ksl = slice(KOFF[g], KOFF[g + 1])
                psk = ps_kvp.tile([128, LK], f32, tag="psk", name="psk")
                nc.tensor.matmul(
                    out=psk[:, 0:kw], lhsT=wkT[:], rhs=xkT[:, ksl],
                    start=True, stop=True, skip_group_check=True,
                )
                kT = kvp.tile([128, LK], bf16, tag="kT", name="kT", bufs=5)
                nc.scalar.activation(
                    out=kT[:, 0:kw], in_=psk[:, 0:kw], func=AFT.Identity)
                psv = ps_kvp.tile([128, LK], f32, tag="psv", name="psv")
                for t in range(KT[g]):
                    nc.tensor.matmul(
                        out=psv[:, t * 128 : (t + 1) * 128],
                        lhsT=xkT[:, KOFF[g] + t * 128 : KOFF[g] + (t + 1) * 128],
                        rhs=wvT[:],
                        start=True, stop=True, skip_group_check=True,
                    )
                v = kvp.tile([128, LK], bf16, tag="v", name="v", bufs=7)
                nc.vector.tensor_copy(out=v[:, 0:kw], in_=psv[:, 0:kw])
                kT_g[g] = kT
                v_g[g] = v

            def emit_lgx(g):
                """logits + exp + exp-sum for slot g"""
                kT = kT_g.pop(g)
                qn = QC[g]
                qsl = slice(QOFF[g], QOFF[g + 1])
                exl = []
                for t in range(KT[g]):
                    lgp = ps_lgp.tile([128, 4 * LQ], f32, tag="lg", name="lgp")
                    nc.tensor.matmul(
                        out=lgp[:, 0 : 4 * qn],
                        lhsT=kT[:, t * 128 : (t + 1) * 128],
                        rhs=qTz[:, :, qsl],
                        start=True, stop=True, skip_group_check=True,
                    )
                    ex = expp.tile([128, 4 * LQ], bf16, tag="ex", name="ex",
                                   bufs=10)
                    nc.scalar.activation(
                        out=ex[:, 0 : 4 * qn], in_=lgp[:, 0 : 4 * qn],
                        func=AFT.Exp)
                    exl.append(ex)
                if KT[g] == 1:
                    exs = exl[0]
                else:
                    exs = expp.tile([128, 4 * LQ], bf16, tag="exs",
                                    name="exs", bufs=4)
                    nc.gpsimd.tensor_add(
                        out=exs[:, 0 : 4 * qn], in0=exl[0][:, 0 : 4 * qn],
                        in1=exl[1][:, 0 : 4 * qn])
                    if KT[g] == 3:
                        nc.vector.tensor_add(
                            out=exs[:, 0 : 4 * qn], in0=exs[:, 0 : 4 * qn],
                            in1=exl[2][:, 0 : 4 * qn])
                ex_g[g] = exl
                exs_g[g] = exs

            def emit_cd(g):
                """ctx + denominator matmuls for slot g"""
                v = v_g.pop(g)
                exl = ex_g.pop(g)
                exs = exs_g.pop(g)
                qn = QC[g]
                att = ps_attp.tile([128, 192], f32, tag="att", name="att")
                for t in range(KT[g]):
                    for h in range(4):
                        nc.tensor.matmul(
                            out=att[32 * h : 32 * (h + 1), 0:qn],
                            lhsT=v[:, t * 128 + 32 * h : t * 128 + 32 * (h + 1)],
                            rhs=exl[t][:, h * qn : (h + 1) * qn],
                            start=(t == 0), stop=(t == KT[g] - 1),
                            skip_group_check=True, tile_position=(0, 32 * h),
                        )
                # denominator, replicated to each head's 32 partitions
                for h in range(4):
                    nc.tensor.matmul(
                        out=att[32 * h : 32 * (h + 1), LQ : LQ + qn],
                        lhsT=ones32[:],
                        rhs=exs[:, h * qn : (h + 1) * qn],
                        start=True, stop=True, skip_group_check=True,
                        tile_position=(0, 32 * h),
                    )
                att_g[g] = att

            def emit_nrm(g):
                """normalize + out-proj + residual for slot g"""
                att = att_g.pop(g)
                qn = QC[g]
                qsl = slice(QOFF[g], QOFF[g + 1])
                dsb = smp.tile([128, LQ], f32, tag="dsb", name="dsb")
                nc.vector.tensor_scalar_add(
                    out=dsb[:, 0:qn], in0=att[:, LQ : LQ + qn],
                    scalar1=negnp[:, g : g + 1],
                )
                rden = smp.tile([128, LQ], f32, tag="rden", name="rden")
                nc.vector.reciprocal_approx_fast(
                    out=rden[:, 0:qn], in_=dsb[:, 0:qn])
                ctxn = smp.tile([128, LQ], bf16, tag="ctxn", name="ctxn")
                nc.vector.tensor_mul(
                    out=ctxn[:, 0:qn], in0=att[:, 0:qn], in1=rden[:, 0:qn])
                po = ps_lgp.tile([128, 4 * LQ], f32, tag="lg", name="po")
                nc.tensor.matmul(
                    out=po[:, 0:qn], lhsT=woT[:], rhs=ctxn[:, 0:qn],
                    start=True, stop=True, skip_group_check=True,
                )
                nc.vector.tensor_add(
                    out=ar[:, qsl], in0=po[:, 0:qn], in1=xqr[:, qsl],
                )

            ffn_state = {}

            def emit_ffn_a(key):
                c0, c1 = key
                n = c1 - c0
                sl = slice(c0, c1)
                arb = ffnp.tile([128, 512], bf16, tag="arb", name="arb")
                nc.vector.tensor_copy(out=arb[:, 0:n], in_=ar[:, sl])
                pa = ps_bigp.tile([128, 512], f32, tag="big", name="pa")
                nc.tensor.matmul(
                    out=pa[:, 0:n], lhsT=w1T[:, 0:128], rhs=arb[:, 0:n],
                    start=True, stop=True, skip_group_check=True,
                )
                ra = ffnp.tile([128, 512], bf16, tag="ra", name="ra")
                nc.scalar.activation(
                    out=ra[:, 0:n], in_=pa[:, 0:n], func=AFT.Relu,
                    bias=b1c[:, 0:1],
                )
                ffn_state[key] = (arb, ra)

            def emit_ffn_b(key, tail=False):
                c0, c1 = key
                n = c1 - c0
                sl = slice(c0, c1)
                arb, ra = ffn_state.pop(key)
                pb = ps_bigp.tile([128, 512], f32, tag="big", name="pb")
                nc.tensor.matmul(
                    out=pb[:, 0:n], lhsT=w1T[:, 128:256], rhs=arb[:, 0:n],
                    start=True, stop=True, skip_group_check=True,
                )
                rb = ffnp.tile([128, 512], bf16, tag="rb", name="rb")
                nc.vector.tensor_scalar(
                    out=rb[:, 0:n], in0=pb[:, 0:n], scalar1=b1c[:, 1:2],
                    scalar2=0.0, op0=ALU.add, op1=ALU.max,
                )
                p2 = ps_bigp.tile([128, 512], f32, tag="big", name="p2")
                nc.tensor.matmul(
                    out=p2[:, 0:n], lhsT=w2T_a[:], rhs=ra[:, 0:n],
                    start=True, stop=False, skip_group_check=True,
                )
                nc.tensor.matmul(
                    out=p2[:, 0:n], lhsT=w2T_b[:], rhs=rb[:, 0:n],
                    start=False, stop=True, skip_group_check=True,
                )
                f2 = ffnp.tile([128, 512], f32, tag="f2", name="f2")
                nc.scalar.activation(
                    out=f2[:, 0:n], in_=p2[:, 0:n], func=AFT.Identity,
                    bias=b2c[:, 0:1],
                )
                if tail:
                    nc.vector.tensor_add(
                        out=ar[:, sl], in0=f2[:, 0:n], in1=ar[:, sl])
                    nc.scalar.dma_start(out=out_d[:, sl], in_=ar[:, sl])
                else:
                    nc.gpsimd.tensor_add(
                        out=ar[:, sl], in0=f2[:, 0:n], in1=ar[:, sl])
                    nc.sync.dma_start(out=out_d[:, sl], in_=ar[:, sl])

            for w in range(n_waves):
                emit_qproj(w)
                if w < G:
                    emit_kv(w)
                if 2 <= w < G + 2:
                    emit_lgx(w - 2)
                if 4 <= w < G + 4:
                    emit_cd(w - 4)
                if 5 <= w < G + 5:
                    emit_nrm(w - 5)
                for key in ffn_a.get(w, ()):
                    emit_ffn_a(key)
                for key in ffn_b.get(w, ()):
                    emit_ffn_b(key, tail=key in tail_keys)
    nc.finalize()
    return nc


_NC_CACHE = {}


def kernel(edge_index, edge_attr, incoming_edges_list, incoming_edges_batch,
           edge_batch, in_proj_w, in_proj_b, out_proj_w, out_proj_b,
           w1, b1, w2, b2):
    global LAST_RESULTS

    edge_attr = np.asarray(edge_attr, np.float32)
    edge_batch = np.asarray(edge_batch, np.int64)
    incoming_edges_list = np.asarray(incoming_edges_list, np.int64)
    incoming_edges_batch = np.asarray(incoming_edges_batch, np.int64)

    cnt_q = np.bincount(edge_batch, minlength=B)
    st_q = np.zeros(B + 1, np.int64)
    np.cumsum(cnt_q, out=st_q[1:])
    cnt_k = np.bincount(incoming_edges_batch, minlength=B)
    st_k = np.zeros(B + 1, np.int64)
    np.cumsum(cnt_k, out=st_k[1:])
    assert cnt_q.max() <= LQ and cnt_k.max() <= LK

    # slot assignment: per core, sort graphs by key count (desc); slot
    # capacities are the across-core maxima at each rank
    perms = np.empty((NCORES, G), np.int64)
    for c in range(NCORES):
        gl = np.arange(c * G, (c + 1) * G)
        perms[c] = gl[np.argsort(-cnt_k[gl], kind="stable")]
    QC = tuple(int(x) for x in (cnt_q[perms].max(axis=0) + 3) // 4 * 4)
    KT = tuple(int(x) for x in
               np.maximum(1, -(-cnt_k[perms].max(axis=0) // 128)))
    slot_of = np.empty(B, np.int64)
    for c in range(NCORES):
        slot_of[perms[c]] = np.arange(G)

    QOFF = np.zeros(G + 1, np.int64)
    np.cumsum(np.array(QC), out=QOFF[1:])
    KOFF = np.zeros(G + 1, np.int64)
    np.cumsum(128 * np.array(KT), out=KOFF[1:])
    QS2, KS2 = int(QOFF[-1]), int(KOFF[-1])

    xpad = np.zeros((E + LQ, H), np.float32)
    xpad[:E] = edge_attr

    s = 1.0 / math.sqrt(HD)
    wq, wk, wv = in_proj_w[:H], in_proj_w[H : 2 * H], in_proj_w[2 * H :]
    bq, bv = in_proj_b[:H], in_proj_b[2 * H :]
    # bk is dropped exactly: softmax is invariant to the per-query shift
    # q.bk added uniformly across a query's keys.
    boc = out_proj_b + out_proj_w @ bv

    wqT = np.ascontiguousarray((wq * s).T, np.float32)
    wqTz = np.zeros((H, 4 * H), np.float32)
    bqz = np.zeros((H, 4), np.float32)
    for h in range(4):
        wqTz[:, h * H + 32 * h : h * H + 32 * (h + 1)] = \
            wqT[:, 32 * h : 32 * (h + 1)]
        bqz[32 * h : 32 * (h + 1), h] = (bq * s)[32 * h : 32 * (h + 1)]

    bft = ml_dtypes.bfloat16
    shared = dict(
        wqTz=np.ascontiguousarray(wqTz.astype(bft)),
        bqz=np.ascontiguousarray(bqz),
        wkT=np.ascontiguousarray(wk.T.astype(bft)),
        wvT=np.ascontiguousarray(wv.T.astype(bft)),
        woT=np.ascontiguousarray(out_proj_w.T.astype(bft)),
        w1T=np.ascontiguousarray(w1.T.astype(bft)),
        w2T=np.ascontiguousarray(w2.T.astype(bft)),
        b1c=np.ascontiguousarray(b1.reshape(2, H).T, np.float32),
        b2c=np.ascontiguousarray(b2[:, None], np.float32),
    )

    in_maps = []
    for c in range(NCORES):
        rows_q = np.empty(QS2, np.int64)
        rows_k = np.empty(KS2, np.int64)
        negnp_c = np.empty(G, np.float32)
        for i in range(G):
            g = perms[c, i]
            rows_q[QOFF[i] : QOFF[i + 1]] = st_q[g] + np.arange(QC[i])
            nk = int(cnt_k[g])
            kcap = 128 * KT[i]
            rk = np.full(kcap, E, np.int64)
            rk[:nk] = incoming_edges_list[st_k[g] : st_k[g] + nk]
            rows_k[KOFF[i] : KOFF[i + 1]] = rk
            negnp_c[i] = -(kcap - nk)
        xq = xpad[rows_q]                                  # [QS2, H] f32
        xk = xpad[rows_k]                                  # [KS2, H] f32
        in_maps.append(dict(
            shared,
            xqr=np.ascontiguousarray(xq.T) + boc[:, None].astype(np.float32),
            xqbf=np.ascontiguousarray(xq.T.astype(bft)),
            xkT=np.ascontiguousarray(xk.T.astype(bft)),
            negnp=np.ascontiguousarray(
                np.broadcast_to(negnp_c, (H, G))),
        ))

    key = (QC, KT)
    if key not in _NC_CACHE:
        _NC_CACHE.clear()
        _NC_CACHE[key] = _build_program(QC, KT)
    res = run_bass_kernel_spmd(
        _NC_CACHE[key], in_maps, core_ids=list(range(NCORES)),
        trace=TRACE, **TRACE_KW,
    )
    LAST_RESULTS = res

    # compact: edge e lives at dense col (QOFF[slot] + pos) of its core
    eb = edge_batch
    pos = np.arange(E) - st_q[eb]
    col = QOFF[slot_of[eb]] + pos
    out_full = np.empty((E, H), np.float32)
    for c in range(NCORES):
        sel = (eb // G) == c
        out_full[sel] = res.results[c]["out"].T[col[sel]]
    return out_full
